# revision 16
# baseline (speedup 1.0000x reference)
"""CompositePerturbation Trainium2 kernel (v2 — transpose-free, bf16 PE).

Pipeline per sample (batch sharded 4-per-core across 8 cores):
  1. Separable 25-tap blur as two banded-matmul passes with the BAND as the
     moving operand, streaming only its ~146 nonzero columns per 128-block:
       pass1: Y^T[w,h] = sum_k X_k[h_in,w]^T-free  x  bandV[h_in, h_out-narrow]
       pass2: Z[h,w']  = sum_j Y^T_j[w,h]-cols     x  bandH[w,  w'-narrow]
     The first matmul of each PSUM tile is full-width (start=True) so the
     2KB zero-region is wholly written; later narrow matmuls accumulate into
     already-written bytes or overwrite still-pending bytes (per-byte
     has_written semantics). No PE transposes, no intermediate transpose
     copies. x/band are bf16 (PSUM accumulates fp32).
  2. Glare + occlusion: single K=2 bf16 rank-2 matmul (gy|-BIG*ro x
     gx|co) injected first (full width) into each pass-2 PSUM tile.
  3. Rain: count = K=60 matmul of exact 0/1 bf16 streak masks; decay
     D = exp(L*count) on ScalarE with per-partition scale L (fp32 exact).
  4. t = clamp01(z)*D + 1 - D  (custom DVE op, PSUM source) -> bf16.
  5. Salt/pepper as a host-precomputed exact trit mask m in {0,1,2} (fp8):
     out = min(max(t, m-1), m) as a single fused custom DVE op.
All image DMA is host-packed to be partition-contiguous; x/out travel bf16,
m travels fp8 -> ~17MB HBM traffic per core.
"""

import numpy as np

B, C, H, W = 32, 3, 512, 512
NCORES = 8
BPC = B // NCORES  # samples per core
KS = 25
HALF = KS // 2  # 12
P = 128
NT = H // P  # 4 partition tiles per image
BIGNEG = 100.0
CW = 3 * W  # 1536

_CACHE = {}

# Narrow output ranges per contraction block k, superset of the true
# support [k*128-12, k*128+140). Offsets MUST stay 32-aligned: unaligned
# partial-PSUM matmul writes hard-wedge the device (NRT_EXEC_UNIT_UNRECOVERABLE).
def _nrange(k):
    return max(0, k * P - 32), min(512, k * P + P + 32)


def _host_params(x, sigma_u, glare_u, occ_u, rain_u, rain_n_u, rain_alpha_u,
                 noise_u, noise_amt_u, apply_flags):
    import ml_dtypes
    f32 = np.float32
    bf16 = ml_dtypes.bfloat16
    fp8 = ml_dtypes.float8_e4m3
    flags = apply_flags.astype(np.int64)

    # ---- blur band tensor [B, 128, 896]: band[r, t] = f(t - 384 - r) ----
    sigma = 1.0 + 3.0 * sigma_u.astype(np.float64)
    coords = np.arange(KS, dtype=np.float64) - HALF
    g = np.exp(-coords[None, :] ** 2 / (2.0 * sigma[:, None] ** 2))
    g = (g / g.sum(axis=1, keepdims=True)).astype(f32)  # [B, 25]
    band = np.zeros((B, P, 896), dtype=f32)
    r = np.arange(P)[:, None]
    t = np.arange(896)[None, :]
    d = t - 384 - r
    inband = np.abs(d) <= HALF
    for b in range(B):
        if flags[b, 0] > 0:
            vals = np.zeros((P, 896), dtype=f32)
            vals[inband] = g[b][(d[inband] + HALF)]
            band[b] = vals
        else:
            band[b] = (d == 0).astype(f32)
    band = band.astype(bf16)

    # ---- glare/occl rank-2 tensor gvt [B, 2, 1024] f32 ----
    yy = np.arange(H, dtype=f32)
    xx = np.arange(W, dtype=f32)
    inten = 0.4 + 0.5 * glare_u[:, 0]
    rx = (0.1 + 0.25 * glare_u[:, 1]) * W / 2
    ry = (0.1 + 0.25 * glare_u[:, 2]) * H / 2
    cx = (0.2 + 0.6 * glare_u[:, 3]) * W
    cy = (0.2 + 0.6 * glare_u[:, 4]) * H
    gy = np.exp(-((yy[None, :] - cy[:, None]) / ry[:, None]) ** 2)
    gx = np.exp(-((xx[None, :] - cx[:, None]) / rx[:, None]) ** 2)
    gy = gy * inten[:, None] * (flags[:, 1] > 0)[:, None]

    ph = np.floor(H * (0.1 + 0.3 * occ_u[:, 0]))
    pw = np.floor(W * (0.1 + 0.3 * occ_u[:, 1]))
    y0 = np.floor(occ_u[:, 2] * (H - ph))
    x0 = np.floor(occ_u[:, 3] * (W - pw))
    ro = ((yy[None, :] >= y0[:, None]) & (yy[None, :] < (y0 + ph)[:, None]))
    co = ((xx[None, :] >= x0[:, None]) & (xx[None, :] < (x0 + pw)[:, None]))
    ro = ro & (flags[:, 2] > 0)[:, None]

    gvt = np.zeros((B, 2, 1024), dtype=f32)
    gvt[:, 0, :512] = gy
    gvt[:, 1, :512] = (-BIGNEG) * ro
    gvt[:, 0, 512:] = gx
    gvt[:, 1, 512:] = co
    gvt = gvt.astype(bf16)

    # ---- rain streak masks (exact 0/1), L applied via activation scale ----
    S = rain_u.shape[1]
    n = np.floor(20.0 + 41.0 * rain_n_u)
    a = 0.15 + 0.35 * rain_alpha_u
    L = np.where(flags[:, 3] > 0, np.log(1.0 - a), 0.0).astype(f32)
    xc = np.floor(rain_u[:, :, 0] * W)
    y0s = np.floor(rain_u[:, :, 1] * (H // 2))
    y1s = (H // 2) + np.floor(rain_u[:, :, 2] * (H // 2))
    hh = np.arange(H, dtype=f32)[None, None, :]
    ww = np.arange(W, dtype=f32)[None, None, :]
    rowm = ((hh >= y0s[:, :, None]) & (hh < y1s[:, :, None]))
    colm = ((ww >= xc[:, :, None] - 1) & (ww <= xc[:, :, None]))
    active = (np.arange(S)[None, :] < n[:, None])
    rn = np.concatenate([rowm & active[:, :, None], colm], axis=2)
    rn = rn.astype(bf16)  # [B, 60, 1024]

    scal = np.zeros((B, P, 2), dtype=f32)
    scal[:, :, 0] = L[:, None]

    # ---- salt/pepper trit mask m in {0,1,2}, exact fp32 compares ----
    amount = (0.01 + 0.07 * noise_amt_u)[:, None, None, None]
    f4 = (flags[:, 4] > 0)[:, None, None, None]
    lo = np.where(f4, amount / 2, 0.0)
    hi = np.where(f4, 1.0 - amount / 2, 2.0)
    trit = (1.0 + (noise_u > hi) - (noise_u < lo)).astype(fp8)
    m = np.ascontiguousarray(
        trit.reshape(B, C, NT, P, W).transpose(0, 3, 2, 1, 4)
    ).reshape(B, P, NT * CW)

    # ---- x packed partition-contiguous bf16 [B, 128, C*2048] ----
    xp = np.ascontiguousarray(
        x.astype(bf16).reshape(B, C, NT, P, W).transpose(0, 3, 1, 2, 4)
    ).reshape(B, P, C * NT * W)

    return xp, m, band, gvt, rn, scal


def _register_dve_ops():
    """Register the fused rain custom-DVE op (documented extension point).

    CPERT_RAIN: out = clamp01(in0) * in1 + 1 - in1
      (in0 = blur+glare-BIG*occl PSUM, in1 = rain decay D)
    """
    from concourse import dve_ops
    from concourse.dve_spec import (
        Spec, Src0, Src1, Zero, One, maxx, minn, lower, _has_src1,
    )
    from concourse.dve_uop import DveOpSpec
    import numpy as np

    if "CPERT_RAIN_ANT" in dve_ops._SUB_OPCODE_FOR_NAME:
        return (dve_ops._BY_NAME_CPERT["CPERT_RAIN_ANT"],
                dve_ops._BY_NAME_CPERT["CPERT_SP2_ANT"])

    def make(name, spec):
        row = dve_ops._CUSTOM_DVE_ROW_BASE + len(dve_ops.OPS)
        assert row < 0x20
        shas = {}
        for ver in ("v3", "v4"):
            tmp = DveOpSpec(name=name, opcode=row, uops=lower(spec, ver=ver),
                            rd1_en=_has_src1(spec))
            shas[ver] = tmp.sha(ver)
        op = dve_ops.DveOp(name, spec, False, shas)
        dve_ops._SUB_OPCODE_FOR_NAME[name] = row
        dve_ops.OPS.append(op)
        dve_ops.CUSTOM_DVE_SPECS[name] = spec
        return op

    rain_spec = Spec(
        body=maxx(minn(Src0, One), Zero) * Src1 + One - Src1,
        reference=lambda in0, in1, s0, s1, imm2: (
            np.clip(in0, 0.0, 1.0).astype(np.float32) * in1 + 1.0 - in1
        ).astype(np.float32),
    )
    sp_spec = Spec(
        body=minn(maxx(Src1, Src0 - One), Src0),
        reference=lambda in0, in1, s0, s1, imm2: np.minimum(
            np.maximum(in1, in0 - 1.0), in0).astype(np.float32),
    )
    rain_op = make("CPERT_RAIN_ANT", rain_spec)
    sp_op = make("CPERT_SP2_ANT", sp_spec)
    dve_ops._BY_NAME_CPERT = {"CPERT_RAIN_ANT": rain_op,
                              "CPERT_SP2_ANT": sp_op}
    return rain_op, sp_op


def _build_module():
    import concourse.bacc as bacc
    import concourse.mybir as mybir
    from concourse.tile import TileContext

    f32 = mybir.dt.float32
    f32r = mybir.dt.float32r
    bf16 = mybir.dt.bfloat16
    fp8 = mybir.dt.float8e4
    AF = mybir.ActivationFunctionType
    OP = mybir.AluOpType

    RAIN_OP, SP_OP = _register_dve_ops()

    nc = bacc.Bacc("TRN2", target_bir_lowering=False, debug=False,
                   num_devices=NCORES)
    x_d = nc.declare_dram_parameter("x", [BPC, P, C * NT * W], bf16, isOutput=False)
    m_d = nc.declare_dram_parameter("m", [BPC, P, NT * CW], fp8, isOutput=False)
    band_d = nc.declare_dram_parameter("band", [BPC, P, 896], bf16, isOutput=False)
    gvt_d = nc.declare_dram_parameter("gvt", [BPC, 2, 1024], bf16, isOutput=False)
    rn_d = nc.declare_dram_parameter("rn", [BPC, 60, 1024], bf16, isOutput=False)
    scal_d = nc.declare_dram_parameter("scal", [BPC, P, 2], f32, isOutput=False)
    out_d = nc.declare_dram_parameter("out", [BPC, NT, P, CW], bf16, isOutput=True)

    with TileContext(nc) as tc:
        with (
            tc.tile_pool(name="params", bufs=2) as ppool,
            tc.tile_pool(name="xin", bufs=2) as xpool,
            tc.tile_pool(name="ytsb", bufs=2) as ytpool,
            tc.tile_pool(name="tcat", bufs=8) as tpool,
            tc.tile_pool(name="ncat", bufs=4) as npool,
            tc.tile_pool(name="dd", bufs=8) as dpool,
            tc.tile_pool(name="sp", bufs=2) as spool,
            tc.tile_pool(name="oc", bufs=2) as opool,
            tc.tile_pool(name="yps", bufs=2, space="PSUM") as ypsum,
            tc.tile_pool(name="zps", bufs=3, space="PSUM") as zpsum,
            tc.tile_pool(name="rps", bufs=2, space="PSUM") as rpsum,
            tc.tile_pool(name="jps", bufs=1, space="PSUM") as jpsum,
        ):
            warmed = False
            for b in range(BPC):
                bandb = ppool.tile([P, 896], bf16, tag="band")
                nc.sync.dma_start(out=bandb[:], in_=band_d[b])
                gvt = ppool.tile([2, 1024], bf16, tag="gvt")
                nc.sync.dma_start(out=gvt[:], in_=gvt_d[b])
                rn = ppool.tile([60, 1024], bf16, tag="rain")
                nc.sync.dma_start(out=rn[:], in_=rn_d[b])
                sc = ppool.tile([P, 2], f32, tag="scal")
                nc.sync.dma_start(out=sc[:], in_=scal_d[b])

                ncat = npool.tile([P, NT * CW], fp8, tag="n")
                nc.sync.dma_start(out=ncat[:], in_=m_d[b])

                if not warmed:
                    # dummy Exp absorbs the ACT table load + bias-const dep
                    warm = ppool.tile([P, 2], f32, tag="warm")
                    nc.scalar.activation(warm[:, 0:1], sc[:, 1:2], AF.Exp)
                    warmed = True

                # PE touch of param tensors: folds their DMA waits into one
                # tiny matmul each so real matmuls stay under the wait limit.
                junk = jpsum.tile([P, 2], f32, tag="psJ", name=f"junk{b}")
                for t_ in (bandb, rn):
                    nc.tensor.matmul(junk[0:1, 0:1], lhsT=t_[0:1, 0:1],
                                     rhs=t_[0:1, 0:1], start=True, stop=True)
                nc.tensor.matmul(junk[0:1, 1:2], lhsT=gvt[0:1, 0:1],
                                 rhs=gvt[0:1, 0:1], start=True, stop=True)

                # ---- rain decay D[u] = exp(L * count) ----
                D_t = []
                for u in range(NT):
                    psA = rpsum.tile([P, W], f32, tag="psA")
                    nc.tensor.matmul(psA[:], lhsT=rn[0:60, u * P:(u + 1) * P],
                                     rhs=rn[0:60, 512:1024],
                                     start=True, stop=True)
                    dt_ = dpool.tile([P, W], f32, tag="D")
                    nc.scalar.activation(dt_[:], psA[:], AF.Exp,
                                         bias=sc[:, 1:2], scale=sc[:, 0:1])
                    D_t.append(dt_)

                tcat = [tpool.tile([P, CW], bf16, tag="t", name=f"tcat{b}_{u}")
                        for u in range(NT)]

                xt_all = xpool.tile([P, C * NT * W], bf16, tag="x")
                nc.sync.dma_start(out=xt_all[:], in_=x_d[b])
                nc.tensor.matmul(junk[0:1, 0:1], lhsT=xt_all[0:1, 0:1],
                                 rhs=xt_all[0:1, 0:1], start=True, stop=True)
                for c in range(C):
                    xt = xt_all[:, c * NT * W:(c + 1) * NT * W]

                    # ---- pass 1: Y^T tiles, band moving w/ narrow columns ----
                    ytsb = ytpool.tile([P, NT * W], bf16, tag="yt")
                    for i in range(NT):
                        psY = ypsum.tile([P, W], f32, tag="psY")
                        # k=0 full width (marks whole zero region written)
                        nc.tensor.matmul(
                            psY[:],
                            lhsT=xt[:, 0 * W + i * P: 0 * W + (i + 1) * P],
                            rhs=bandb[:, 384:896],
                            start=True, stop=False)
                        for k in range(1, NT):
                            c0, c1 = _nrange(k)
                            nc.tensor.matmul(
                                psY[:, c0:c1],
                                lhsT=xt[:, k * W + i * P: k * W + (i + 1) * P],
                                rhs=bandb[:, 384 + c0 - k * P: 384 + c1 - k * P],
                                start=False, stop=(k == NT - 1))
                        nc.scalar.copy(ytsb[:, i * W:(i + 1) * W], psY[:])

                    # ---- pass 2 + glare/occl inject + rain DVE op ----
                    for u in range(NT):
                        psZ = zpsum.tile([P, W], f32, tag="psZ")
                        nc.tensor.matmul(psZ[:], lhsT=gvt[:, u * P:(u + 1) * P],
                                         rhs=gvt[:, 512:1024],
                                         start=True, stop=False)
                        for j in range(NT):
                            c0, c1 = _nrange(j)
                            nc.tensor.matmul(
                                psZ[:, c0:c1],
                                lhsT=ytsb[:, j * W + u * P: j * W + (u + 1) * P],
                                rhs=bandb[:, 384 + c0 - j * P: 384 + c1 - j * P],
                                start=False, stop=(j == NT - 1))
                        # t = clamp01(z) * D + 1 - D   (fused custom DVE op)
                        nc.vector._custom_dve(
                            RAIN_OP,
                            out=tcat[u][:, c * W:(c + 1) * W],
                            in0=psZ[:], in1=D_t[u][:],
                        )

                # ---- salt/pepper via trit mask on GpSimd, store ----
                for u in range(NT):
                    ocat = opool.tile([P, CW], bf16, tag="o")
                    nc.vector._custom_dve(
                        SP_OP, out=ocat[:], in0=ncat[:, u * CW:(u + 1) * CW], in1=tcat[u][:],
                    )
                    nc.sync.dma_start(out=out_d[b, u], in_=ocat[:])
    nc.finalize()
    return nc


def _get_module():
    if "nc" not in _CACHE:
        _CACHE["nc"] = _build_module()
    return _CACHE["nc"]


def kernel(**inputs):
    x = np.asarray(inputs["x"], dtype=np.float32)
    noise = np.asarray(inputs["noise_u"], dtype=np.float32)
    xp, m, band, gvt, rn, scal = _host_params(
        x, np.asarray(inputs["sigma_u"]), np.asarray(inputs["glare_u"]),
        np.asarray(inputs["occ_u"]), np.asarray(inputs["rain_u"]),
        np.asarray(inputs["rain_n_u"]), np.asarray(inputs["rain_alpha_u"]),
        noise, np.asarray(inputs["noise_amt_u"]),
        np.asarray(inputs["apply_flags"]),
    )

    from concourse.bass_utils import run_bass_kernel_spmd

    nc = _get_module()
    in_maps = []
    for i in range(NCORES):
        s = slice(i * BPC, (i + 1) * BPC)
        in_maps.append({
            "x": np.ascontiguousarray(xp[s]),
            "m": np.ascontiguousarray(m[s]),
            "band": np.ascontiguousarray(band[s]),
            "gvt": np.ascontiguousarray(gvt[s]),
            "rn": np.ascontiguousarray(rn[s]),
            "scal": np.ascontiguousarray(scal[s]),
        })
    import os
    trace_env = os.environ.get("CPERT_TRACE", "")
    kw = {}
    if trace_env:
        kw["trace"] = True
        kw["trace_cores"] = [int(c) for c in trace_env.split(",")]
    res = run_bass_kernel_spmd(nc, in_maps, list(range(NCORES)), **kw)
    if trace_env:
        _CACHE["last_results"] = res
    o = np.concatenate([r["out"] for r in res.results], axis=0)  # [B,NT,P,CW]
    o = o.reshape(B, NT, P, C, W).transpose(0, 3, 1, 2, 4).reshape(B, C, H, W)
    return np.ascontiguousarray(o).astype(np.float32)


# revision 17
# speedup vs baseline: 1.0049x; 1.0049x over previous
"""CompositePerturbation Trainium2 kernel (v2 — transpose-free, bf16 PE).

Pipeline per sample (batch sharded 4-per-core across 8 cores):
  1. Separable 25-tap blur as two banded-matmul passes with the BAND as the
     moving operand, streaming only its ~146 nonzero columns per 128-block:
       pass1: Y^T[w,h] = sum_k X_k[h_in,w]^T-free  x  bandV[h_in, h_out-narrow]
       pass2: Z[h,w']  = sum_j Y^T_j[w,h]-cols     x  bandH[w,  w'-narrow]
     The first matmul of each PSUM tile is full-width (start=True) so the
     2KB zero-region is wholly written; later narrow matmuls accumulate into
     already-written bytes or overwrite still-pending bytes (per-byte
     has_written semantics). No PE transposes, no intermediate transpose
     copies. x/band are bf16 (PSUM accumulates fp32).
  2. Glare + occlusion: single K=2 bf16 rank-2 matmul (gy|-BIG*ro x
     gx|co) injected first (full width) into each pass-2 PSUM tile.
  3. Rain: count = K=60 matmul of exact 0/1 bf16 streak masks; decay
     D = exp(L*count) on ScalarE with per-partition scale L (fp32 exact).
  4. t = clamp01(z)*D + 1 - D  (custom DVE op, PSUM source) -> bf16.
  5. Salt/pepper as a host-precomputed exact trit mask m in {0,1,2} (fp8):
     out = min(max(t, m-1), m) as a single fused custom DVE op.
All image DMA is host-packed to be partition-contiguous; x/out travel bf16,
m travels fp8 -> ~17MB HBM traffic per core.
"""

import numpy as np

B, C, H, W = 32, 3, 512, 512
NCORES = 8
BPC = B // NCORES  # samples per core
KS = 25
HALF = KS // 2  # 12
P = 128
NT = H // P  # 4 partition tiles per image
BIGNEG = 100.0
CW = 3 * W  # 1536

_CACHE = {}

# Narrow output ranges per contraction block k, superset of the true
# support [k*128-12, k*128+140). Offsets MUST stay 32-aligned: unaligned
# partial-PSUM matmul writes hard-wedge the device (NRT_EXEC_UNIT_UNRECOVERABLE).
def _nrange(k):
    return max(0, k * P - 32), min(512, k * P + P + 32)


def _host_params(x, sigma_u, glare_u, occ_u, rain_u, rain_n_u, rain_alpha_u,
                 noise_u, noise_amt_u, apply_flags):
    import ml_dtypes
    f32 = np.float32
    bf16 = ml_dtypes.bfloat16
    fp8 = ml_dtypes.float8_e4m3
    flags = apply_flags.astype(np.int64)

    # ---- blur band tensor [B, 128, 896]: band[r, t] = f(t - 384 - r) ----
    sigma = 1.0 + 3.0 * sigma_u.astype(np.float64)
    coords = np.arange(KS, dtype=np.float64) - HALF
    g = np.exp(-coords[None, :] ** 2 / (2.0 * sigma[:, None] ** 2))
    g = (g / g.sum(axis=1, keepdims=True)).astype(f32)  # [B, 25]
    band = np.zeros((B, P, 896), dtype=f32)
    r = np.arange(P)[:, None]
    t = np.arange(896)[None, :]
    d = t - 384 - r
    inband = np.abs(d) <= HALF
    for b in range(B):
        if flags[b, 0] > 0:
            vals = np.zeros((P, 896), dtype=f32)
            vals[inband] = g[b][(d[inband] + HALF)]
            band[b] = vals
        else:
            band[b] = (d == 0).astype(f32)
    band = band.astype(bf16)

    # ---- glare/occl rank-2 tensor gvt [B, 2, 1024] f32 ----
    yy = np.arange(H, dtype=f32)
    xx = np.arange(W, dtype=f32)
    inten = 0.4 + 0.5 * glare_u[:, 0]
    rx = (0.1 + 0.25 * glare_u[:, 1]) * W / 2
    ry = (0.1 + 0.25 * glare_u[:, 2]) * H / 2
    cx = (0.2 + 0.6 * glare_u[:, 3]) * W
    cy = (0.2 + 0.6 * glare_u[:, 4]) * H
    gy = np.exp(-((yy[None, :] - cy[:, None]) / ry[:, None]) ** 2)
    gx = np.exp(-((xx[None, :] - cx[:, None]) / rx[:, None]) ** 2)
    gy = gy * inten[:, None] * (flags[:, 1] > 0)[:, None]

    ph = np.floor(H * (0.1 + 0.3 * occ_u[:, 0]))
    pw = np.floor(W * (0.1 + 0.3 * occ_u[:, 1]))
    y0 = np.floor(occ_u[:, 2] * (H - ph))
    x0 = np.floor(occ_u[:, 3] * (W - pw))
    ro = ((yy[None, :] >= y0[:, None]) & (yy[None, :] < (y0 + ph)[:, None]))
    co = ((xx[None, :] >= x0[:, None]) & (xx[None, :] < (x0 + pw)[:, None]))
    ro = ro & (flags[:, 2] > 0)[:, None]

    gvt = np.zeros((B, 2, 1024), dtype=f32)
    gvt[:, 0, :512] = gy
    gvt[:, 1, :512] = (-BIGNEG) * ro
    gvt[:, 0, 512:] = gx
    gvt[:, 1, 512:] = co
    gvt = gvt.astype(bf16)

    # ---- rain streak masks (exact 0/1), L applied via activation scale ----
    S = rain_u.shape[1]
    n = np.floor(20.0 + 41.0 * rain_n_u)
    a = 0.15 + 0.35 * rain_alpha_u
    L = np.where(flags[:, 3] > 0, np.log(1.0 - a), 0.0).astype(f32)
    xc = np.floor(rain_u[:, :, 0] * W)
    y0s = np.floor(rain_u[:, :, 1] * (H // 2))
    y1s = (H // 2) + np.floor(rain_u[:, :, 2] * (H // 2))
    hh = np.arange(H, dtype=f32)[None, None, :]
    ww = np.arange(W, dtype=f32)[None, None, :]
    rowm = ((hh >= y0s[:, :, None]) & (hh < y1s[:, :, None]))
    colm = ((ww >= xc[:, :, None] - 1) & (ww <= xc[:, :, None]))
    active = (np.arange(S)[None, :] < n[:, None])
    rn = np.concatenate([rowm & active[:, :, None], colm], axis=2)
    rn = rn.astype(bf16)  # [B, 60, 1024]

    scal = np.zeros((B, P, 2), dtype=f32)
    scal[:, :, 0] = L[:, None]

    # ---- salt/pepper trit mask m in {0,1,2}, exact fp32 compares ----
    amount = (0.01 + 0.07 * noise_amt_u)[:, None, None, None]
    f4 = (flags[:, 4] > 0)[:, None, None, None]
    lo = np.where(f4, amount / 2, 0.0)
    hi = np.where(f4, 1.0 - amount / 2, 2.0)
    trit = (1.0 + (noise_u > hi) - (noise_u < lo)).astype(fp8)
    m = np.ascontiguousarray(
        trit.reshape(B, C, NT, P, W).transpose(0, 3, 2, 1, 4)
    ).reshape(B, P, NT * CW)

    # ---- x packed partition-contiguous bf16 [B, 128, C*2048] ----
    xp = np.ascontiguousarray(
        x.astype(bf16).reshape(B, C, NT, P, W).transpose(0, 3, 1, 2, 4)
    ).reshape(B, P, C * NT * W)

    return xp, m, band, gvt, rn, scal


def _register_dve_ops():
    """Register the fused rain custom-DVE op (documented extension point).

    CPERT_RAIN: out = clamp01(in0) * in1 + 1 - in1
      (in0 = blur+glare-BIG*occl PSUM, in1 = rain decay D)
    """
    from concourse import dve_ops
    from concourse.dve_spec import (
        Spec, Src0, Src1, Zero, One, maxx, minn, lower, _has_src1,
    )
    from concourse.dve_uop import DveOpSpec
    import numpy as np

    if "CPERT_RAIN_ANT" in dve_ops._SUB_OPCODE_FOR_NAME:
        return (dve_ops._BY_NAME_CPERT["CPERT_RAIN_ANT"],
                dve_ops._BY_NAME_CPERT["CPERT_SP2_ANT"])

    def make(name, spec):
        row = dve_ops._CUSTOM_DVE_ROW_BASE + len(dve_ops.OPS)
        assert row < 0x20
        shas = {}
        for ver in ("v3", "v4"):
            tmp = DveOpSpec(name=name, opcode=row, uops=lower(spec, ver=ver),
                            rd1_en=_has_src1(spec))
            shas[ver] = tmp.sha(ver)
        op = dve_ops.DveOp(name, spec, False, shas)
        dve_ops._SUB_OPCODE_FOR_NAME[name] = row
        dve_ops.OPS.append(op)
        dve_ops.CUSTOM_DVE_SPECS[name] = spec
        return op

    rain_spec = Spec(
        body=maxx(minn(Src0, One), Zero) * Src1 + One - Src1,
        reference=lambda in0, in1, s0, s1, imm2: (
            np.clip(in0, 0.0, 1.0).astype(np.float32) * in1 + 1.0 - in1
        ).astype(np.float32),
    )
    sp_spec = Spec(
        body=minn(maxx(Src1, Src0 - One), Src0),
        reference=lambda in0, in1, s0, s1, imm2: np.minimum(
            np.maximum(in1, in0 - 1.0), in0).astype(np.float32),
    )
    rain_op = make("CPERT_RAIN_ANT", rain_spec)
    sp_op = make("CPERT_SP2_ANT", sp_spec)
    dve_ops._BY_NAME_CPERT = {"CPERT_RAIN_ANT": rain_op,
                              "CPERT_SP2_ANT": sp_op}
    return rain_op, sp_op


def _build_module():
    import concourse.bacc as bacc
    import concourse.mybir as mybir
    from concourse.tile import TileContext

    f32 = mybir.dt.float32
    f32r = mybir.dt.float32r
    bf16 = mybir.dt.bfloat16
    fp8 = mybir.dt.float8e4
    AF = mybir.ActivationFunctionType
    OP = mybir.AluOpType

    RAIN_OP, SP_OP = _register_dve_ops()

    nc = bacc.Bacc("TRN2", target_bir_lowering=False, debug=False,
                   num_devices=NCORES)
    x_d = nc.declare_dram_parameter("x", [BPC, P, C * NT * W], bf16, isOutput=False)
    m_d = nc.declare_dram_parameter("m", [BPC, P, NT * CW], fp8, isOutput=False)
    band_d = nc.declare_dram_parameter("band", [BPC, P, 896], bf16, isOutput=False)
    gvt_d = nc.declare_dram_parameter("gvt", [BPC, 2, 1024], bf16, isOutput=False)
    rn_d = nc.declare_dram_parameter("rn", [BPC, 60, 1024], bf16, isOutput=False)
    scal_d = nc.declare_dram_parameter("scal", [BPC, P, 2], f32, isOutput=False)
    out_d = nc.declare_dram_parameter("out", [BPC, NT, P, CW], bf16, isOutput=True)

    with TileContext(nc) as tc:
        with (
            tc.tile_pool(name="params", bufs=2) as ppool,
            tc.tile_pool(name="xin", bufs=2) as xpool,
            tc.tile_pool(name="ytsb", bufs=2) as ytpool,
            tc.tile_pool(name="tcat", bufs=8) as tpool,
            tc.tile_pool(name="ncat", bufs=4) as npool,
            tc.tile_pool(name="dd", bufs=8) as dpool,
            tc.tile_pool(name="sp", bufs=2) as spool,
            tc.tile_pool(name="oc", bufs=2) as opool,
            tc.tile_pool(name="yps", bufs=2, space="PSUM") as ypsum,
            tc.tile_pool(name="zps", bufs=3, space="PSUM") as zpsum,
            tc.tile_pool(name="rps", bufs=2, space="PSUM") as rpsum,
            tc.tile_pool(name="jps", bufs=1, space="PSUM") as jpsum,
        ):
            warmed = False
            for b in range(BPC):
                bandb = ppool.tile([P, 896], bf16, tag="band")
                nc.sync.dma_start(out=bandb[:], in_=band_d[b])
                gvt = ppool.tile([2, 1024], bf16, tag="gvt")
                nc.sync.dma_start(out=gvt[:], in_=gvt_d[b])
                rn = ppool.tile([60, 1024], bf16, tag="rain")
                nc.sync.dma_start(out=rn[:], in_=rn_d[b])
                sc = ppool.tile([P, 2], f32, tag="scal")
                nc.sync.dma_start(out=sc[:], in_=scal_d[b])

                ncat = npool.tile([P, NT * CW], fp8, tag="n")
                nc.sync.dma_start(out=ncat[:], in_=m_d[b])

                if not warmed:
                    # dummy Exp absorbs the ACT table load + bias-const dep
                    warm = ppool.tile([P, 2], f32, tag="warm")
                    nc.scalar.activation(warm[:, 0:1], sc[:, 1:2], AF.Exp)
                    warmed = True

                # PE touch of param tensors: folds their DMA waits into one
                # tiny matmul each so real matmuls stay under the wait limit.
                junk = jpsum.tile([P, 2], f32, tag="psJ", name=f"junk{b}")
                for t_ in (bandb, rn):
                    nc.tensor.matmul(junk[0:1, 0:1], lhsT=t_[0:1, 0:1],
                                     rhs=t_[0:1, 0:1], start=True, stop=True)
                nc.tensor.matmul(junk[0:1, 1:2], lhsT=gvt[0:1, 0:1],
                                 rhs=gvt[0:1, 0:1], start=True, stop=True)

                # ---- rain decay D[u] = exp(L * count) ----
                D_t = []
                for u in range(NT):
                    psA = rpsum.tile([P, W], f32, tag="psA")
                    nc.tensor.matmul(psA[:], lhsT=rn[0:60, u * P:(u + 1) * P],
                                     rhs=rn[0:60, 512:1024],
                                     start=True, stop=True)
                    dt_ = dpool.tile([P, W], f32, tag="D")
                    nc.scalar.activation(dt_[:], psA[:], AF.Exp,
                                         bias=sc[:, 1:2], scale=sc[:, 0:1])
                    D_t.append(dt_)

                tcat = [tpool.tile([P, CW], bf16, tag="t", name=f"tcat{b}_{u}")
                        for u in range(NT)]

                xt_all = xpool.tile([P, C * NT * W], bf16, tag="x")
                nc.sync.dma_start(out=xt_all[:], in_=x_d[b])
                nc.tensor.matmul(junk[0:1, 0:1], lhsT=xt_all[0:1, 0:1],
                                 rhs=xt_all[0:1, 0:1], start=True, stop=True)
                for c in range(C):
                    xt = xt_all[:, c * NT * W:(c + 1) * NT * W]

                    # ---- pass 1: Y^T tiles, band moving w/ narrow columns ----
                    ytsb = ytpool.tile([P, NT * W], bf16, tag="yt")
                    for i in range(NT):
                        psY = ypsum.tile([P, W], f32, tag="psY")
                        # all-narrow: k=0 start=True marks the bank pending;
                        # later matmuls accumulate written bytes, overwrite
                        # still-pending ones (per-byte has_written)
                        for k in range(NT):
                            c0, c1 = _nrange(k)
                            nc.tensor.matmul(
                                psY[:, c0:c1],
                                lhsT=xt[:, k * W + i * P: k * W + (i + 1) * P],
                                rhs=bandb[:, 384 + c0 - k * P: 384 + c1 - k * P],
                                start=(k == 0), stop=(k == NT - 1))
                        nc.scalar.copy(ytsb[:, i * W:(i + 1) * W], psY[:])

                    # ---- pass 2 + glare/occl inject + rain DVE op ----
                    for u in range(NT):
                        psZ = zpsum.tile([P, W], f32, tag="psZ")
                        nc.tensor.matmul(psZ[:], lhsT=gvt[:, u * P:(u + 1) * P],
                                         rhs=gvt[:, 512:1024],
                                         start=True, stop=False)
                        for j in range(NT):
                            c0, c1 = _nrange(j)
                            nc.tensor.matmul(
                                psZ[:, c0:c1],
                                lhsT=ytsb[:, j * W + u * P: j * W + (u + 1) * P],
                                rhs=bandb[:, 384 + c0 - j * P: 384 + c1 - j * P],
                                start=False, stop=(j == NT - 1))
                        # t = clamp01(z) * D + 1 - D   (fused custom DVE op)
                        nc.vector._custom_dve(
                            RAIN_OP,
                            out=tcat[u][:, c * W:(c + 1) * W],
                            in0=psZ[:], in1=D_t[u][:],
                        )

                # ---- salt/pepper via trit mask on GpSimd, store ----
                for u in range(NT):
                    ocat = opool.tile([P, CW], bf16, tag="o")
                    nc.vector._custom_dve(
                        SP_OP, out=ocat[:], in0=ncat[:, u * CW:(u + 1) * CW], in1=tcat[u][:],
                    )
                    nc.sync.dma_start(out=out_d[b, u], in_=ocat[:])
    nc.finalize()
    return nc


def _get_module():
    if "nc" not in _CACHE:
        _CACHE["nc"] = _build_module()
    return _CACHE["nc"]


def kernel(**inputs):
    x = np.asarray(inputs["x"], dtype=np.float32)
    noise = np.asarray(inputs["noise_u"], dtype=np.float32)
    xp, m, band, gvt, rn, scal = _host_params(
        x, np.asarray(inputs["sigma_u"]), np.asarray(inputs["glare_u"]),
        np.asarray(inputs["occ_u"]), np.asarray(inputs["rain_u"]),
        np.asarray(inputs["rain_n_u"]), np.asarray(inputs["rain_alpha_u"]),
        noise, np.asarray(inputs["noise_amt_u"]),
        np.asarray(inputs["apply_flags"]),
    )

    from concourse.bass_utils import run_bass_kernel_spmd

    nc = _get_module()
    in_maps = []
    for i in range(NCORES):
        s = slice(i * BPC, (i + 1) * BPC)
        in_maps.append({
            "x": np.ascontiguousarray(xp[s]),
            "m": np.ascontiguousarray(m[s]),
            "band": np.ascontiguousarray(band[s]),
            "gvt": np.ascontiguousarray(gvt[s]),
            "rn": np.ascontiguousarray(rn[s]),
            "scal": np.ascontiguousarray(scal[s]),
        })
    import os
    trace_env = os.environ.get("CPERT_TRACE", "")
    kw = {}
    if trace_env:
        kw["trace"] = True
        kw["trace_cores"] = [int(c) for c in trace_env.split(",")]
    res = run_bass_kernel_spmd(nc, in_maps, list(range(NCORES)), **kw)
    if trace_env:
        _CACHE["last_results"] = res
    o = np.concatenate([r["out"] for r in res.results], axis=0)  # [B,NT,P,CW]
    o = o.reshape(B, NT, P, C, W).transpose(0, 3, 1, 2, 4).reshape(B, C, H, W)
    return np.ascontiguousarray(o).astype(np.float32)
